# revision 12
# baseline (speedup 1.0000x reference)
"""CLIPtrase recalibration kernel for Trainium2 (Bass/Tile).

Per-batch computation (B=8, one batch element per NeuronCore):
    x  : (2304, 768) f32
    xn = x / ||x||_row
    S  = xn @ xn.T              (cosine correlation, symmetric)
    W  = softmax(S / 0.05, axis=-1)
    out = 0.5 * x + 0.5 * (W @ x)

Layout strategy (keys-on-partition):
    - xT: xn transposed into [d-partition, n-free] bf16 layout, built with PE
      transposes. Both QK operands come from xT.
    - S is computed as S^T[j_keys(part), i_queries(free)] per 512-query
      megablock; since S is symmetric this equals S[i, j] as needed.
    - exp fused on ACT: E = exp(20*S - 20). The softmax row max is exactly
      cos(i,i)=1 (scaled: 20), so bias=-20 replaces max-subtraction.
    - E^T tiles are directly the stationary operand of the PV matmul
      (no transposition of the attention weights needed).
    - PV rhs = [x | 1] bf16: the ones column makes PSUM col 768 accumulate
      the softmax denominator alongside recal in cols 0:768.
"""

import math
import sys

sys.path.insert(0, "/opt/trn_rl_repo")

import numpy as np

import concourse.bass as bass
import concourse.mybir as mybir
from concourse import bacc
from concourse.bass_utils import run_bass_kernel_spmd
from concourse.masks import make_identity
from concourse.tile import TileContext

F32 = mybir.dt.float32
BF16 = mybir.dt.bfloat16

B = 8
H = 48
W = 48
N = H * W          # 2304
D = 768
P = 128
NT = N // P        # 18 row tiles
DT = D // P        # 6 feature tiles
TEMP_INV = 20.0    # 1 / 0.05
# query megablocks: 4 x 512 + 1 x 256
MEGAS = [(0, 512), (512, 512), (1024, 512), (1536, 512), (2048, 256)]

_CACHED = {}


def build_program():
    nc = bacc.Bacc()
    x_in = nc.declare_dram_parameter("x", [N, D], F32, isOutput=False)
    out_dram = nc.declare_dram_parameter("out", [N, D], F32, isOutput=True)

    with TileContext(nc) as tc:
        with (
            tc.tile_pool(name="persist", bufs=1) as persist,
            tc.tile_pool(name="work", bufs=3) as work,
            tc.tile_pool(name="et_pool", bufs=2) as et_pool,
            tc.tile_pool(name="psS", bufs=2, space="PSUM") as psum_s,
            tc.tile_pool(name="psPV", bufs=2, space="PSUM") as psum_pv,
            tc.tile_pool(name="psT", bufs=2, space="PSUM") as psum_t,
        ):
            # persistent tensors
            x_full = persist.tile([P, NT, D], F32)    # raw x, resident
            xT = persist.tile([P, DT, N], BF16)       # xn^T  [d, n]
            x_aug = persist.tile([P, NT, D + 1], BF16)  # [x | 1] by row tile
            ident = persist.tile([P, P], BF16)
            make_identity(nc, ident)
            # ones column for the softmax denominator
            nc.vector.memset(x_aug[:, :, D : D + 1], 1.0)
            # constant bias APs for ACT instructions
            zero_bias = persist.tile([P, 1], F32)
            nc.vector.memset(zero_bias, 0.0)
            negtemp = persist.tile([P, 1], F32)
            nc.vector.memset(negtemp, -TEMP_INV)

            # ---- Phase 1: load, row norms, xn, transpose to xT ----
            for j in range(NT):
                xt = x_full[:, j, :]
                nc.sync.dma_start(out=xt, in_=x_in[j * P : (j + 1) * P, :])

                # mean/var in 3 subgroups of 256 -> sumsq = D*(var + mean^2)
                stats = work.tile([P, 3, 6], F32, tag="stats")
                for g in range(3):
                    nc.vector.bn_stats(
                        out=stats[:, g, :], in_=xt[:, g * 256 : (g + 1) * 256]
                    )
                mv = work.tile([P, 2], F32, tag="mv")
                nc.vector.bn_aggr(out=mv, in_=stats)
                msq = work.tile([P, 1], F32, tag="msq")
                nc.vector.tensor_mul(msq, mv[:, 0:1], mv[:, 0:1])
                ssum = work.tile([P, 1], F32, tag="ssum")
                nc.vector.tensor_add(ssum, mv[:, 1:2], msq)
                # norm = sqrt(D * ssum); inv = 1/norm
                nrm = work.tile([P, 1], F32, tag="nrm")
                nc.scalar.activation(
                    nrm,
                    ssum,
                    mybir.ActivationFunctionType.Sqrt,
                    bias=zero_bias,
                    scale=float(D),
                )
                inv = work.tile([P, 1], F32, tag="inv")
                nc.vector.reciprocal(inv, nrm)

                # x_aug (bf16 copy of raw x)
                nc.vector.tensor_copy(x_aug[:, j, 0:D], xt)
                # xn = x * inv_norm  (bf16)
                xn = work.tile([P, D], BF16, tag="xn")
                nc.vector.tensor_scalar_mul(xn, xt, inv)
                # transpose xn into xT columns j*P:(j+1)*P
                for d in range(DT):
                    pt = psum_t.tile([P, P], BF16, tag="pt")
                    nc.tensor.transpose(pt, xn[:, d * P : (d + 1) * P], ident)
                    nc.vector.tensor_copy(xT[:, d, j * P : (j + 1) * P], pt)

            # ---- Phase 2+3 per query megablock ----
            for q0, qw in MEGAS:
                et = et_pool.tile([P, NT, 512], BF16, tag="ET")
                for j in range(NT):
                    ps = psum_s.tile([P, 512], F32, tag="psS")
                    for d in range(DT):
                        nc.tensor.matmul(
                            ps[:, :qw],
                            lhsT=xT[:, d, j * P : (j + 1) * P],
                            rhs=xT[:, d, q0 : q0 + qw],
                            start=(d == 0),
                            stop=(d == DT - 1),
                        )
                    # E^T = exp(20*S - 20)
                    nc.scalar.activation(
                        et[:, j, :qw],
                        ps[:, :qw],
                        mybir.ActivationFunctionType.Exp,
                        bias=negtemp,
                        scale=TEMP_INV,
                    )

                for isub in range(qw // P):
                    iblk = q0 // P + isub  # global row-tile index
                    pv = psum_pv.tile([P, 1024], F32, tag="psPV")
                    for j in range(NT):
                        lhsT = et[:, j, isub * P : (isub + 1) * P]
                        nc.tensor.matmul(
                            pv[:, 0:512],
                            lhsT=lhsT,
                            rhs=x_aug[:, j, 0:512],
                            start=(j == 0),
                            stop=(j == NT - 1),
                        )
                        nc.tensor.matmul(
                            pv[:, 512 : D + 1],
                            lhsT=lhsT,
                            rhs=x_aug[:, j, 512 : D + 1],
                            start=(j == 0),
                            stop=(j == NT - 1),
                        )
                    # blend: out = 0.5*x + (0.5/sum) * recal
                    inv2 = work.tile([P, 1], F32, tag="inv2")
                    nc.vector.reciprocal(inv2, pv[:, D : D + 1])
                    invh = work.tile([P, 1], F32, tag="invh")
                    nc.vector.tensor_scalar_mul(invh, inv2, 0.5)
                    t = work.tile([P, D], F32, tag="t")
                    nc.vector.tensor_scalar_mul(t, pv[:, 0:D], invh)
                    ot = work.tile([P, D], F32, tag="ot")
                    nc.vector.scalar_tensor_tensor(
                        ot,
                        in0=x_full[:, iblk, :],
                        scalar=0.5,
                        in1=t,
                        op0=mybir.AluOpType.mult,
                        op1=mybir.AluOpType.add,
                    )
                    nc.gpsimd.dma_start(
                        out=out_dram[iblk * P : (iblk + 1) * P, :], in_=ot
                    )

    if not nc.is_finalized():
        nc.finalize()
    return nc


def _get_program():
    if "nc" not in _CACHED:
        _CACHED["nc"] = build_program()
    return _CACHED["nc"]


def kernel(**inputs):
    features = inputs["features"]
    assert features.shape == (B, H, W, D), features.shape
    x = np.ascontiguousarray(features.reshape(B, N, D)).astype(np.float32)
    nc = _get_program()
    in_maps = [{"x": x[b]} for b in range(B)]
    res = run_bass_kernel_spmd(nc, in_maps, core_ids=list(range(B)))
    out = np.stack([res.results[b]["out"] for b in range(B)], axis=0)
    return out.reshape(B, H, W, D).astype(np.float32)


# revision 14
# speedup vs baseline: 1.0464x; 1.0464x over previous
"""CLIPtrase recalibration kernel for Trainium2 (Bass/Tile).

Per-batch computation (B=8, one batch element per NeuronCore):
    x  : (2304, 768) f32
    xn = x / ||x||_row
    S  = xn @ xn.T              (cosine correlation, symmetric)
    W  = softmax(S / 0.05, axis=-1)
    out = 0.5 * x + 0.5 * (W @ x)

Layout strategy (keys-on-partition, symmetric-S reuse):
    - xT: xn transposed into [d-partition, n-free] bf16 layout, built with PE
      transposes. Both QK operands come from xT.
    - S^T[a-keys(part), b-queries(free)] is computed per 512-query megablock
      m, but only for key tiles a <= amax(m) (upper parallelogram). exp is
      fused on ACT: E = exp(20*S - 20); the softmax row max is exactly
      cos(i,i)=1 (scaled: 20), so bias=-20 replaces max-subtraction. All E
      blocks stay resident in SBUF (bf16, ~49KB/partition).
    - E is symmetric, so the PV stationary blocks missing from the upper
      parallelogram are PE transposes (128 cols) of stored blocks instead of
      full recomputation (768 cols of matmul) -- a ~40% QK saving.
    - PV rhs = [x | 1] bf16: the ones column makes PSUM col 768 accumulate
      the softmax denominator alongside recal in cols 0:768.
"""

import sys

sys.path.insert(0, "/opt/trn_rl_repo")

import numpy as np

import concourse.bass as bass
import concourse.mybir as mybir
from concourse import bacc
from concourse.bass_utils import run_bass_kernel_spmd
from concourse.masks import make_identity
from concourse.tile import TileContext

F32 = mybir.dt.float32
BF16 = mybir.dt.bfloat16

B = 8
H = 48
W = 48
N = H * W          # 2304
D = 768
P = 128
NT = N // P        # 18 row tiles
DT = D // P        # 6 feature tiles
TEMP_INV = 20.0    # 1 / 0.05
# query megablocks: (start, width, n key tiles computed)
MEGAS = [(0, 512), (512, 512), (1024, 512), (1536, 512), (2048, 256)]

_CACHED = {}


def build_program():
    nc = bacc.Bacc()
    x_in = nc.declare_dram_parameter("x", [N, D], F32, isOutput=False)
    out_dram = nc.declare_dram_parameter("out", [N, D], F32, isOutput=True)

    # mega index and amax (last key-tile computed) per mega
    mega_of_block = {}
    amax = []
    for mi, (q0, qw) in enumerate(MEGAS):
        for b in range(q0 // P, (q0 + qw) // P):
            mega_of_block[b] = mi
        amax.append((q0 + qw) // P - 1)

    with TileContext(nc) as tc:
        with (
            tc.tile_pool(name="persist", bufs=1) as persist,
            tc.tile_pool(name="work", bufs=3) as work,
            tc.tile_pool(name="estore", bufs=1) as estore,
            tc.tile_pool(name="psS", bufs=2, space="PSUM") as psum_s,
            tc.tile_pool(name="psPV", bufs=2, space="PSUM") as psum_pv,
            tc.tile_pool(name="psT", bufs=2, space="PSUM") as psum_t,
        ):
            # persistent tensors
            x_full = persist.tile([P, NT, D], F32)      # raw x, resident
            xT = persist.tile([P, DT, N], BF16)         # xn^T  [d, n]
            x_aug = persist.tile([P, NT, D + 1], BF16)  # [x | 1] by row tile
            ident = persist.tile([P, P], BF16)
            make_identity(nc, ident)
            nc.vector.memset(x_aug[:, :, D : D + 1], 1.0)
            zero_bias = persist.tile([P, 1], F32)
            nc.vector.memset(zero_bias, 0.0)
            negtemp = persist.tile([P, 1], F32)
            nc.vector.memset(negtemp, -TEMP_INV)

            # E storage: per mega m, E^T[a, b-cols] for a <= amax(m)
            e_tiles = [
                estore.tile(
                    [P, amax[mi] + 1, qw], BF16, tag=f"E{mi}", name=f"E{mi}"
                )
                for mi, (q0, qw) in enumerate(MEGAS)
            ]

            # ---- Phase 1: load, row norms, xn, transpose to xT ----
            for j in range(NT):
                xt = x_full[:, j, :]
                nc.sync.dma_start(out=xt, in_=x_in[j * P : (j + 1) * P, :])

                # sum of squares in one pass: (x*1)*x with accumulate
                scratch = work.tile([P, D], BF16, tag="scratch")
                ssum = work.tile([P, 1], F32, tag="ssum")
                nc.vector.scalar_tensor_tensor(
                    scratch,
                    in0=xt,
                    scalar=1.0,
                    in1=xt,
                    op0=mybir.AluOpType.mult,
                    op1=mybir.AluOpType.mult,
                    accum_out=ssum,
                )
                # norm = sqrt(ssum); inv = 1/norm
                nrm = work.tile([P, 1], F32, tag="nrm")
                nc.scalar.activation(
                    nrm, ssum, mybir.ActivationFunctionType.Sqrt, bias=zero_bias
                )
                inv = work.tile([P, 1], F32, tag="inv")
                nc.vector.reciprocal(inv, nrm)

                # x_aug (bf16 copy of raw x)
                nc.vector.tensor_copy(x_aug[:, j, 0:D], xt)
                # xn = x * inv_norm  (bf16)
                xn = work.tile([P, D], BF16, tag="xn")
                nc.vector.tensor_scalar_mul(xn, xt, inv)
                # transpose xn into xT columns j*P:(j+1)*P
                for d in range(DT):
                    pt = psum_t.tile([P, P], BF16, tag="pt")
                    nc.tensor.transpose(pt, xn[:, d * P : (d + 1) * P], ident)
                    nc.vector.tensor_copy(xT[:, d, j * P : (j + 1) * P], pt)

            # ---- Phase 2: QK upper parallelogram + exp ----
            for mi, (q0, qw) in enumerate(MEGAS):
                et = e_tiles[mi]
                for a in range(amax[mi] + 1):
                    ps = psum_s.tile([P, 512], F32, tag="psS")
                    for d in range(DT):
                        nc.tensor.matmul(
                            ps[:, :qw],
                            lhsT=xT[:, d, a * P : (a + 1) * P],
                            rhs=xT[:, d, q0 : q0 + qw],
                            start=(d == 0),
                            stop=(d == DT - 1),
                        )
                    # E^T = exp(20*S - 20)
                    nc.scalar.activation(
                        et[:, a, :qw],
                        ps[:, :qw],
                        mybir.ActivationFunctionType.Exp,
                        bias=negtemp,
                        scale=TEMP_INV,
                    )

            def e_block(a, b):
                """AP of stored E^T[a-tile, b-tile cols] (only if a<=amax)."""
                mi = mega_of_block[b]
                q0 = MEGAS[mi][0]
                off = b * P - q0
                return e_tiles[mi][:, a, off : off + P]

            # ---- Phase 3: PV + blend per query block b ----
            for b in range(NT):
                mb = mega_of_block[b]
                pv = psum_pv.tile([P, 1024], F32, tag="psPV")
                for a in range(NT):
                    if a <= amax[mb]:
                        lhsT = e_block(a, b)
                    else:
                        # E^T[a,b] = transpose(E^T[b,a]) by symmetry of E
                        ptb = psum_t.tile([P, P], BF16, tag="pt")
                        nc.tensor.transpose(ptb, e_block(b, a), ident)
                        stg = work.tile([P, P], BF16, tag="etT")
                        nc.vector.tensor_copy(stg, ptb)
                        lhsT = stg
                    nc.tensor.matmul(
                        pv[:, 0:512],
                        lhsT=lhsT,
                        rhs=x_aug[:, a, 0:512],
                        start=(a == 0),
                        stop=(a == NT - 1),
                    )
                    nc.tensor.matmul(
                        pv[:, 512 : D + 1],
                        lhsT=lhsT,
                        rhs=x_aug[:, a, 512 : D + 1],
                        start=(a == 0),
                        stop=(a == NT - 1),
                    )
                # blend: out = 0.5*x + (0.5/sum) * recal
                inv2 = work.tile([P, 1], F32, tag="inv2")
                nc.vector.reciprocal(inv2, pv[:, D : D + 1])
                invh = work.tile([P, 1], F32, tag="invh")
                nc.vector.tensor_scalar_mul(invh, inv2, 0.5)
                t = work.tile([P, D], F32, tag="t")
                nc.vector.tensor_scalar_mul(t, pv[:, 0:D], invh)
                ot = work.tile([P, D], F32, tag="ot")
                nc.vector.scalar_tensor_tensor(
                    ot,
                    in0=x_full[:, b, :],
                    scalar=0.5,
                    in1=t,
                    op0=mybir.AluOpType.mult,
                    op1=mybir.AluOpType.add,
                )
                nc.gpsimd.dma_start(
                    out=out_dram[b * P : (b + 1) * P, :], in_=ot
                )

    if not nc.is_finalized():
        nc.finalize()
    return nc


def _get_program():
    if "nc" not in _CACHED:
        _CACHED["nc"] = build_program()
    return _CACHED["nc"]


def kernel(**inputs):
    features = inputs["features"]
    assert features.shape == (B, H, W, D), features.shape
    x = np.ascontiguousarray(features.reshape(B, N, D)).astype(np.float32)
    nc = _get_program()
    in_maps = [{"x": x[b]} for b in range(B)]
    res = run_bass_kernel_spmd(nc, in_maps, core_ids=list(range(B)))
    out = np.stack([res.results[b]["out"] for b in range(B)], axis=0)
    return out.reshape(B, H, W, D).astype(np.float32)


# revision 16
# speedup vs baseline: 1.1095x; 1.0603x over previous
"""CLIPtrase recalibration kernel for Trainium2 (Bass/Tile).

Per-batch computation (B=8, one batch element per NeuronCore):
    x  : (2304, 768) f32
    xn = x / ||x||_row
    S  = xn @ xn.T              (cosine correlation, symmetric)
    W  = softmax(S / 0.05, axis=-1)
    out = 0.5 * x + 0.5 * (W @ x)

Layout strategy (keys-on-partition, symmetric-S reuse):
    - xT: xn transposed into [d-partition, n-free] bf16 layout, built with PE
      transposes. Both QK operands come from xT.
    - S^T[a-keys(part), b-queries(free)] is computed per 512-query megablock
      m, but only for key tiles a <= amax(m) (upper parallelogram). exp is
      fused on ACT: E = exp(20*S - 20); the softmax row max is exactly
      cos(i,i)=1 (scaled: 20), so bias=-20 replaces max-subtraction. All E
      blocks stay resident in SBUF (bf16, ~49KB/partition).
    - E is symmetric, so the PV stationary blocks missing from the upper
      parallelogram are PE transposes (128 cols) of stored blocks instead of
      full recomputation (768 cols of matmul) -- a ~40% QK saving.
    - PV rhs = [x | 1] bf16: the ones column makes PSUM col 768 accumulate
      the softmax denominator alongside recal in cols 0:768.
"""

import sys

sys.path.insert(0, "/opt/trn_rl_repo")

import numpy as np

import concourse.bass as bass
import concourse.mybir as mybir
from concourse import bacc
from concourse.bass_utils import run_bass_kernel_spmd
from concourse.masks import make_identity
from concourse.tile import TileContext

F32 = mybir.dt.float32
BF16 = mybir.dt.bfloat16

B = 8
H = 48
W = 48
N = H * W          # 2304
D = 768
P = 128
NT = N // P        # 18 row tiles
DT = D // P        # 6 feature tiles
TEMP_INV = 20.0    # 1 / 0.05
# query megablocks: (start, width, n key tiles computed)
MEGAS = [(0, 512), (512, 512), (1024, 512), (1536, 512), (2048, 256)]

_CACHED = {}


def build_program():
    nc = bacc.Bacc()
    x_in = nc.declare_dram_parameter("x", [N, D], F32, isOutput=False)
    out_dram = nc.declare_dram_parameter("out", [N, D], F32, isOutput=True)

    # mega index and amax (last key-tile computed) per mega
    mega_of_block = {}
    amax = []
    for mi, (q0, qw) in enumerate(MEGAS):
        for b in range(q0 // P, (q0 + qw) // P):
            mega_of_block[b] = mi
        amax.append((q0 + qw) // P - 1)

    with TileContext(nc) as tc:
        with (
            tc.tile_pool(name="persist", bufs=1) as persist,
            tc.tile_pool(name="work", bufs=3) as work,
            tc.tile_pool(name="estore", bufs=1) as estore,
            tc.tile_pool(name="psS", bufs=2, space="PSUM") as psum_s,
            tc.tile_pool(name="psPV", bufs=2, space="PSUM") as psum_pv,
            tc.tile_pool(name="psT", bufs=2, space="PSUM") as psum_t,
        ):
            # persistent tensors
            x_full = persist.tile([P, NT, D], F32)      # raw x, resident
            xT = persist.tile([P, DT, N], BF16)         # xn^T  [d, n]
            x_aug = persist.tile([P, NT, D + 1], BF16)  # [x | 1] by row tile
            ident = persist.tile([P, P], BF16)
            make_identity(nc, ident)
            nc.vector.memset(x_aug[:, :, D : D + 1], 1.0)
            zero_bias = persist.tile([P, 1], F32)
            nc.vector.memset(zero_bias, 0.0)
            negtemp = persist.tile([P, 1], F32)
            nc.vector.memset(negtemp, -TEMP_INV)

            # E storage: per mega m, E^T[a, b-cols] for a <= amax(m)
            e_tiles = [
                estore.tile(
                    [P, amax[mi] + 1, qw], BF16, tag=f"E{mi}", name=f"E{mi}"
                )
                for mi, (q0, qw) in enumerate(MEGAS)
            ]

            # ---- Phase 1: load, row norms, xn, transpose to xT ----
            for j in range(NT):
                xt = x_full[:, j, :]
                # two half-width DMAs on different queues halve arrival latency
                nc.sync.dma_start(
                    out=x_full[:, j, 0 : D // 2],
                    in_=x_in[j * P : (j + 1) * P, 0 : D // 2],
                )
                nc.sync.dma_start(
                    out=x_full[:, j, D // 2 : D],
                    in_=x_in[j * P : (j + 1) * P, D // 2 : D],
                )

                # sum of squares in one pass: (x*1)*x with accumulate
                scratch = work.tile([P, D], BF16, tag="scratch")
                ssum = work.tile([P, 1], F32, tag="ssum")
                nc.vector.scalar_tensor_tensor(
                    scratch,
                    in0=xt,
                    scalar=1.0,
                    in1=xt,
                    op0=mybir.AluOpType.mult,
                    op1=mybir.AluOpType.mult,
                    accum_out=ssum,
                )
                # norm = sqrt(ssum); inv = 1/norm
                nrm = work.tile([P, 1], F32, tag="nrm")
                nc.scalar.activation(
                    nrm, ssum, mybir.ActivationFunctionType.Sqrt, bias=zero_bias
                )
                inv = work.tile([P, 1], F32, tag="inv")
                nc.vector.reciprocal(inv, nrm)

                # x_aug (bf16 copy of raw x)
                nc.vector.tensor_copy(x_aug[:, j, 0:D], xt)
                # xn = x * inv_norm  (bf16)
                xn = work.tile([P, D], BF16, tag="xn")
                nc.vector.tensor_scalar_mul(xn, xt, inv)
                # transpose xn into xT columns j*P:(j+1)*P
                for d in range(DT):
                    pt = psum_t.tile([P, P], BF16, tag="pt")
                    nc.tensor.transpose(pt, xn[:, d * P : (d + 1) * P], ident)
                    nc.vector.tensor_copy(xT[:, d, j * P : (j + 1) * P], pt)

            # ---- Phase 2: QK upper parallelogram + exp ----
            for mi, (q0, qw) in enumerate(MEGAS):
                et = e_tiles[mi]
                for a in range(amax[mi] + 1):
                    ps = psum_s.tile([P, 512], F32, tag="psS")
                    for d in range(DT):
                        nc.tensor.matmul(
                            ps[:, :qw],
                            lhsT=xT[:, d, a * P : (a + 1) * P],
                            rhs=xT[:, d, q0 : q0 + qw],
                            start=(d == 0),
                            stop=(d == DT - 1),
                        )
                    # E^T = exp(20*S - 20)
                    nc.scalar.activation(
                        et[:, a, :qw],
                        ps[:, :qw],
                        mybir.ActivationFunctionType.Exp,
                        bias=negtemp,
                        scale=TEMP_INV,
                    )

            def e_block(a, b):
                """AP of stored E^T[a-tile, b-tile cols] (only if a<=amax)."""
                mi = mega_of_block[b]
                q0 = MEGAS[mi][0]
                off = b * P - q0
                return e_tiles[mi][:, a, off : off + P]

            # ---- Phase 3: PV + blend per query block b ----
            for b in range(NT):
                mb = mega_of_block[b]
                # stage transposed stationaries (a > amax) in groups of 4:
                # 4 PE transposes into one PSUM bank, one DVE copy out.
                staged = {}
                miss = list(range(amax[mb] + 1, NT))
                for g0 in range(0, len(miss), 4):
                    grp = miss[g0 : g0 + 4]
                    ptb = psum_t.tile([P, 512], BF16, tag="pt")
                    for k, a in enumerate(grp):
                        nc.tensor.transpose(
                            ptb[:, k * P : (k + 1) * P], e_block(b, a), ident
                        )
                    stg = work.tile([P, 512], BF16, tag="etT")
                    nc.vector.tensor_copy(
                        stg[:, : len(grp) * P], ptb[:, : len(grp) * P]
                    )
                    for k, a in enumerate(grp):
                        staged[a] = stg[:, k * P : (k + 1) * P]

                pv = psum_pv.tile([P, 1024], F32, tag="psPV")
                for a in range(NT):
                    lhsT = e_block(a, b) if a <= amax[mb] else staged[a]
                    nc.tensor.matmul(
                        pv[:, 0:512],
                        lhsT=lhsT,
                        rhs=x_aug[:, a, 0:512],
                        start=(a == 0),
                        stop=(a == NT - 1),
                    )
                    nc.tensor.matmul(
                        pv[:, 512 : D + 1],
                        lhsT=lhsT,
                        rhs=x_aug[:, a, 512 : D + 1],
                        start=(a == 0),
                        stop=(a == NT - 1),
                    )
                # blend: out = 0.5*x + (0.5/sum) * recal
                inv2 = work.tile([P, 1], F32, tag="inv2")
                nc.vector.reciprocal(inv2, pv[:, D : D + 1])
                invh = work.tile([P, 1], F32, tag="invh")
                nc.vector.tensor_scalar_mul(invh, inv2, 0.5)
                # t = recal * (0.5/sum) on ACT (idle during PV phase)
                t = work.tile([P, D], F32, tag="t")
                nc.scalar.mul(t, pv[:, 0:D], invh)
                ot = work.tile([P, D], F32, tag="ot")
                for h in range(2):
                    sl = slice(h * (D // 2), (h + 1) * (D // 2))
                    nc.vector.scalar_tensor_tensor(
                        ot[:, sl],
                        in0=x_full[:, b, sl],
                        scalar=0.5,
                        in1=t[:, sl],
                        op0=mybir.AluOpType.mult,
                        op1=mybir.AluOpType.add,
                    )
                    nc.gpsimd.dma_start(
                        out=out_dram[b * P : (b + 1) * P, sl], in_=ot[:, sl]
                    )

    if not nc.is_finalized():
        nc.finalize()
    return nc


def _get_program():
    if "nc" not in _CACHED:
        _CACHED["nc"] = build_program()
    return _CACHED["nc"]


def kernel(**inputs):
    features = inputs["features"]
    assert features.shape == (B, H, W, D), features.shape
    x = np.ascontiguousarray(features.reshape(B, N, D)).astype(np.float32)
    nc = _get_program()
    in_maps = [{"x": x[b]} for b in range(B)]
    res = run_bass_kernel_spmd(nc, in_maps, core_ids=list(range(B)))
    out = np.stack([res.results[b]["out"] for b in range(B)], axis=0)
    return out.reshape(B, H, W, D).astype(np.float32)
